# revision 24
# baseline (speedup 1.0000x reference)
"""Multi-head attention (B=4, S=1024, H=1024, heads=16) on 8 trn2 NeuronCores.

Sharding: data-parallel over batch (4) x tensor-parallel over head-groups (2).
Core c handles batch c//2, heads [8*(c%2), 8*(c%2)+8).

Per-core kernel (all matmuls bf16 with fp32 PSUM accumulation):
  - q/k projections produce qh/kh in [d, i] layout (head-pair dim on
    partitions), one tile per i-half so consumers start on half-finished
    projections; the attention scale 1/8 is applied by exp (scale=0.125).
  - attention runs HEAD-GRANULAR: per head, 8 key-blocks of
    [scores matmul -> exp (ScalarE) -> *exp(bias) (VectorE/GpSimdE)].
    Head granularity needs only 2 PSUM banks for the ctx accumulators, so
    the scores PSUM ring gets 3 buffers (6 banks) and the tensor engine can
    run blocks ahead of exp without stalling.
  - ctx_T plus the softmax denominator come from ONE matmul per (jb, i-half):
    stationary = [vh | ones] (65 cols); a global queue trails the scores
    stream by ~6 blocks and crosses head boundaries.
  - normalization per (head, i-half): the raw denominator row is copied to
    SBUF (bf16), partition-broadcast into PSUM with a ones-column matmul,
    reciprocal'd (custom DVE op), then multiplied into ctxn. Each head's
    normalize fires as soon as its last ctx matmul is emitted, recycling the
    2-buffer cr ring in time for the next head.
  - output projection row-parallel bf16; partials stored bf16, summed (+bo)
    on the host.

Scheduling: pair 0's projection runs up front; pairs 1-3 and the v
projection drain as ~1-2us chunks at fixed block slots in the attention
stream. Low-priority input loads (expb head, v, wo) are dependency-gated
behind the q/k projections so the q/k DMA stream gets full bandwidth.
"""

import numpy as np
import ml_dtypes

BF16 = ml_dtypes.bfloat16

S = 1024
HID = 1024
GCOL = 512  # hidden cols per core (8 heads * 64)
DH = 64
P = 128
NPAIR = 4  # head pairs per core
NJB = 8  # key blocks of 128
NCB = 8  # contraction blocks of 128
NIB = 8  # query blocks of 128

CTX_LAG = 6  # blocks the ctx matmuls trail the scores stream
E_BUFS = 14
EB_BUFS = 14

_CACHED_NC = None


def _build_nc():
    import concourse.bass as bass
    import concourse.mybir as mybir
    import concourse.tile as tile
    from concourse import bacc
    from contextlib import ExitStack

    f32 = mybir.dt.float32
    bf16 = mybir.dt.bfloat16
    AF = mybir.ActivationFunctionType

    nc = bacc.Bacc(
        "TRN2",
        target_bir_lowering=False,
        debug=False,
        enable_asserts=False,
        num_devices=8,
    )

    qT = nc.dram_tensor("qT", [HID, S], bf16, kind="ExternalInput").ap()
    kT = nc.dram_tensor("kT", [HID, S], bf16, kind="ExternalInput").ap()
    vT = nc.dram_tensor("vT", [HID, S], bf16, kind="ExternalInput").ap()
    wq = nc.dram_tensor("wq", [HID, GCOL], bf16, kind="ExternalInput").ap()
    wk = nc.dram_tensor("wk", [HID, GCOL], bf16, kind="ExternalInput").ap()
    wv = nc.dram_tensor("wv", [HID, GCOL], bf16, kind="ExternalInput").ap()
    wo = nc.dram_tensor("wo", [GCOL, HID], bf16, kind="ExternalInput").ap()
    bq = nc.dram_tensor("bq", [GCOL], f32, kind="ExternalInput").ap()
    bk = nc.dram_tensor("bk", [GCOL], f32, kind="ExternalInput").ap()
    bv = nc.dram_tensor("bv", [GCOL], bf16, kind="ExternalInput").ap()
    expb = nc.dram_tensor("expb", [8, S, S], bf16, kind="ExternalInput").ap()
    out = nc.dram_tensor("out", [S, HID], bf16, kind="ExternalOutput").ap()

    with tile.TileContext(nc) as tc, ExitStack() as ctx:
        const = ctx.enter_context(tc.tile_pool(name="const", bufs=1))
        inT = ctx.enter_context(tc.tile_pool(name="inT", bufs=1))
        proj = ctx.enter_context(tc.tile_pool(name="proj", bufs=1))
        work = ctx.enter_context(tc.tile_pool(name="work", bufs=2))
        psum = ctx.enter_context(tc.tile_pool(name="psum", bufs=3, space="PSUM"))

        # ---- constants / weights ----
        wq_sb = const.tile([P, NCB, GCOL], bf16, tag="wq")
        wk_sb = const.tile([P, NCB, GCOL], bf16, tag="wk")
        wv_sb = const.tile([P, NCB, GCOL], bf16, tag="wv")
        wo_sb = const.tile([P, NPAIR, HID], bf16, tag="wo")
        wq_r = wq.rearrange("(cb p) n -> p cb n", p=P)
        wk_r = wk.rearrange("(cb p) n -> p cb n", p=P)
        wv_r = wv.rearrange("(cb p) n -> p cb n", p=P)
        bq_sb = const.tile([P, NPAIR], f32, tag="bq")
        bk_sb = const.tile([P, NPAIR], f32, tag="bk")
        nc.sync.dma_start(out=bq_sb, in_=bq.rearrange("(pr p) -> p pr", p=P))
        nc.sync.dma_start(out=bk_sb, in_=bk.rearrange("(pr p) -> p pr", p=P))
        bv_sb = const.tile([1, GCOL], bf16, tag="bv")
        nc.sync.dma_start(out=bv_sb, in_=bv.rearrange("(a n) -> a n", a=1))
        ones_k1 = const.tile([1, P], bf16, tag="ones_k1")
        nc.vector.memset(ones_k1, 1.0)
        ones_bc = const.tile([1, DH], bf16, tag="ones_bc")
        nc.vector.memset(ones_bc, 1.0)
        wup_sb = const.tile([1, GCOL], bf16, tag="wup")
        nc.vector.memset(wup_sb, 1.0)
        gp_w = const.tile([1, GCOL], bf16, tag="gp_w")
        nc.gpsimd.memset(gp_w, 1.0)

        # projections, one tile per (pair, i-half)
        qh2 = [[proj.tile([P, GCOL], bf16, name=f"qh{i}_{c}", tag=f"qh{i}_{c}")
                for c in range(2)] for i in range(NPAIR)]
        kh2 = [[proj.tile([P, GCOL], bf16, name=f"kh{i}_{c}", tag=f"kh{i}_{c}")
                for c in range(2)] for i in range(NPAIR)]
        # vh_sb[jb]: [j in block, head, 65] where col 64 is ones (denominator)
        vh_sb = [proj.tile([P, 8, DH + 1], bf16, name=f"vh{i}", tag=f"vh{i}")
                 for i in range(NJB)]
        ctxn = [proj.tile([P, S], bf16, name=f"ctxn{i}", tag=f"ctxn{i}")
                for i in range(NPAIR)]

        # ---- PE/ACT warmup during initial DMA wait ----
        wup_ps = [psum.tile([P, GCOL], f32, name=f"wup{w}", tag="cr", bufs=2)
                  for w in range(2)]
        for w in range(4):
            nc.tensor.matmul(wup_ps[w % 2], lhsT=ones_k1, rhs=wup_sb,
                             start=True, stop=True)
        es_w = work.tile([P, S], bf16, name="es_w", tag="es", bufs=3)
        nc.scalar.activation(es_w[:, 0:GCOL], wup_ps[1], AF.Exp, scale=0.125)
        rbc_w = work.tile([DH + 1, GCOL], f32, name="rbc_w", tag="rbc", bufs=2)
        nc.vector.memset(rbc_w, 1.0)
        nc.vector.reciprocal_approx_fast(rbc_w[0:DH, :], rbc_w[0:DH, :])

        # ---- q/k input + weight loads (per-cb tiles pipeline the proj) ----
        qk_tiles = {}
        nc.sync.dma_start(out=wq_sb, in_=wq_r)
        nc.sync.dma_start(out=wk_sb, in_=wk_r)
        for tname, srcT in (("q", qT), ("k", kT)):
            tl = []
            for cb in range(NCB):
                t = inT.tile([P, S], bf16, name=f"{tname}T{cb}", tag="inT",
                             bufs=16)
                nc.sync.dma_start(out=t, in_=srcT[cb * P:(cb + 1) * P, :])
                tl.append(t)
            qk_tiles[tname] = tl

        def qk_half(tname, pr, ic):
            """One half-projection chunk: 8 matmuls + bias-add to bf16."""
            w_sb, b_sb, dst2 = (
                (wq_sb, bq_sb, qh2) if tname == "q" else (wk_sb, bk_sb, kh2)
            )
            pp = psum.tile([P, GCOL], f32, name=f"pp{tname}{pr}_{ic}", tag="mm")
            for cb in range(NCB):
                nc.tensor.matmul(
                    pp,
                    lhsT=w_sb[:, cb, pr * P:(pr + 1) * P],
                    rhs=qk_tiles[tname][cb][:, ic * 512:(ic + 1) * 512],
                    start=(cb == 0),
                    stop=(cb == NCB - 1),
                )
            nc.vector.tensor_scalar_add(dst2[pr][ic], pp, b_sb[:, pr:pr + 1])

        def v_half(jb, gh):
            """Project j-block jb for heads [4*gh, 4*gh+4)."""
            gsl = slice(gh * 256, (gh + 1) * 256)
            ps = psum.tile([P, 256], f32, name=f"vp{jb}_{gh}", tag="mm")
            for cb in range(NCB):
                nc.tensor.matmul(
                    ps,
                    lhsT=v_sb[:, cb, jb * P:(jb + 1) * P],
                    rhs=wv_sb[:, cb, gsl],
                    start=(cb == 0),
                    stop=False,
                )
            nc.tensor.matmul(ps, lhsT=ones_k1, rhs=bv_sb[:, gsl],
                             start=False, stop=True)
            nc.vector.tensor_copy(
                out=vh_sb[jb][:, 4 * gh:4 * gh + 4, 0:DH],
                in_=ps.rearrange("p (h d) -> p h d", d=DH),
            )
            if gh == 1:
                nc.vector.memset(vh_sb[jb][:, :, DH:DH + 1], 1.0)

        # ---- q projections for all pairs run up front: they consume qT
        # tiles as they arrive and fill the otherwise DMA-bound head phase --
        for ic in range(2):
            qk_half("q", 0, ic)

        # DMA rings serve all pending triggers concurrently, so emission
        # order alone cannot prioritize the q/k load stream. Gate the
        # lower-priority loads with a tiny gpsimd read of (target, qh/kh):
        # the load's write-after-read dep then holds it until q/k are done.
        gate_sb = const.tile([1, 4], bf16, tag="gate")

        def gate(target_slice, on):
            nc.vector.memset(target_slice, 0.0)
            nc.gpsimd.tensor_mul(gate_sb, target_slice, on[0:1, 0:4])

        for pr in (1, 2, 3):
            for ic in range(2):
                qk_half("q", pr, ic)

        # ---- first 8 expb block loads (gated on pair-0 q projection) ----
        eb_pre = []
        for bi in range(8):
            eb = work.tile([P, S], bf16, name=f"ebp{bi}", tag="eb",
                           bufs=EB_BUFS)
            gate(eb[0:1, 0:4], qh2[3][1])
            nc.sync.dma_start(out=eb, in_=expb[0, bi * P:(bi + 1) * P, :])
            eb_pre.append(eb)

        # ---- pair 0's k projection (starts right as kT finishes loading) --
        for ic in range(2):
            qk_half("k", 0, ic)

        # ---- v/wo loads, gated on pair-0 k projection ----
        gate(wv_sb[0:1, 0, 0:4], kh2[0][1])
        nc.sync.dma_start(out=wv_sb, in_=wv_r)
        v_sb = inT.tile([P, NCB, S], bf16, name="vT", tag="vT", bufs=1)
        gate(v_sb[0:1, 0, 0:4], kh2[0][1])
        nc.sync.dma_start(out=v_sb, in_=vT.rearrange("(cb p) n -> p cb n", p=P))
        gate(wo_sb[0:1, 0, 0:4], kh2[0][1])
        nc.sync.dma_start(out=wo_sb,
                          in_=wo.rearrange("(pr p) n -> p pr n", p=P))

        # ---- chunk drain plan: global block index -> emitters ----
        drain_plan = {}
        for i, (pr, ic) in enumerate(
            (p, c) for p in (1, 2, 3) for c in range(2)
        ):
            drain_plan.setdefault(i, []).append(
                lambda pr=pr, ic=ic: qk_half("k", pr, ic)
            )
        # v drains: vh[jb] ready by the end of block 8+jb
        for jb in range(NJB):
            for gh in range(2):
                drain_plan.setdefault(8 + jb, []).append(
                    lambda jb=jb, gh=gh: v_half(jb, gh)
                )

        # ---- normalize ----
        def _norm_mul(h, ic, cr_ic, rbc):
            pr, hl = divmod(h, 2)
            if hl == 0:
                nc.vector.tensor_mul(
                    ctxn[pr][0:DH, ic * 512:(ic + 1) * 512],
                    cr_ic[0:DH, :],
                    rbc,
                )
            else:
                ch = work.tile([DH, GCOL], bf16, name=f"ch{h}_{ic}", tag="ch",
                               bufs=2)
                nc.vector.tensor_mul(ch, cr_ic[0:DH, :], rbc)
                nc.gpsimd.dma_start(
                    out=ctxn[pr][DH:2 * DH, ic * 512:(ic + 1) * 512], in_=ch
                )

        def normalize_mm(h, ic, cr_ic):
            """Same, but partition-broadcast via a ones-column matmul (fast
            chain, used for the latency-critical last head)."""
            r_sb = work.tile([1, GCOL], bf16, name=f"r{h}_{ic}", tag="rrow",
                             bufs=2)
            nc.vector.tensor_copy(r_sb, cr_ic[DH:DH + 1, :])
            rbcp = psum.tile([DH, GCOL], f32, name=f"rp{h}_{ic}", tag="mm")
            nc.tensor.matmul(rbcp, lhsT=ones_bc, rhs=r_sb, start=True, stop=True)
            rbc = work.tile([DH + 1, GCOL], f32, name=f"rb{h}_{ic}",
                            tag="rbc", bufs=2)
            nc.vector.reciprocal_approx_fast(rbc[0:DH, :], rbcp)
            _norm_mul(h, ic, cr_ic, rbc[0:DH, :])

        # ---- attention: head-granular with a global lagged ctx queue ----
        cr_of = {}
        cr_queue = []

        HEAD_ORDER = [0, 1, 2, 3, 4, 5, 6, 7]

        def emit_cr(h, jb, e):
            if h not in cr_of:
                cr_of[h] = [
                    psum.tile([DH + 1, GCOL], f32, name=f"cr{h}_{c}",
                              tag="cr", bufs=2)
                    for c in range(2)
                ]
            for ic in range(2):
                nc.tensor.matmul(
                    cr_of[h][ic],
                    lhsT=vh_sb[jb][:, h, :],
                    rhs=e[:, ic * 512:(ic + 1) * 512],
                    start=(jb == 0),
                    stop=(jb == NJB - 1),
                )

        pending_norm = []

        def try_pops(bi):
            pops = 0
            while cr_queue and len(cr_queue) > CTX_LAG and pops < 2:
                h, jb, e = cr_queue[0]
                # first head's ctx must wait for its vh drain (end of blk 8+jb)
                if h == HEAD_ORDER[0] and bi < 9 + jb:
                    break
                emit_cr(*cr_queue.pop(0))
                pops += 1
                if jb == NJB - 1 and h != HEAD_ORDER[-1]:
                    # head h's ctx is complete; its normalize (and with it
                    # the cr-ring recycle) can start now
                    pending_norm.extend([(h, 0), (h, 1)])

        for hi, h in enumerate(HEAD_ORDER):
            pr, hl = divmod(h, 2)
            for jb in range(NJB):
                bi = 8 * hi + jb
                if bi < 8:
                    eb = eb_pre[bi]
                else:
                    eb = work.tile([P, S], bf16, name=f"eb{h}_{jb}", tag="eb",
                                   bufs=EB_BUFS)
                    nc.sync.dma_start(out=eb,
                                      in_=expb[h, jb * P:(jb + 1) * P, :])
                s_ps = psum.tile([P, S], f32, name=f"s{h}_{jb}", tag="mm")
                jbh, jr = divmod(jb, 4)
                for ic in range(2):
                    nc.tensor.matmul(
                        s_ps[:, ic * 512:(ic + 1) * 512],
                        lhsT=kh2[pr][jbh][hl * DH:(hl + 1) * DH,
                                          jr * P:(jr + 1) * P],
                        rhs=qh2[pr][ic][hl * DH:(hl + 1) * DH, :],
                        start=True,
                        stop=True,
                    )
                es = work.tile([P, S], bf16, name=f"es{h}_{jb}", tag="es",
                               bufs=3)
                nc.scalar.activation(es, s_ps, AF.Exp, scale=0.125)
                e = work.tile([P, S], bf16, name=f"e{h}_{jb}", tag="e",
                              bufs=E_BUFS)
                if bi % 4 == 2:
                    nc.gpsimd.tensor_mul(e, es, eb)
                else:
                    nc.vector.tensor_mul(e, es, eb)
                cr_queue.append((h, jb, e))
                try_pops(bi)
                if pending_norm:
                    hn, icn = pending_norm.pop(0)
                    normalize_mm(hn, icn, cr_of[hn][icn])
                for fn in drain_plan.pop(bi, ()):
                    fn()

        # flush remaining ctx + pending norms
        while cr_queue:
            emit_cr(*cr_queue.pop(0))
        for hn, icn in pending_norm:
            normalize_mm(hn, icn, cr_of[hn][icn])

        # ---- output projection (head 7 norm interleaved: ic0, ib0-3, ic1) --
        def outproj(ib):
            yp = psum.tile([P, HID], f32, name=f"yp{ib}", tag="mm")
            for pr in range(NPAIR):
                for cc in range(2):
                    nc.tensor.matmul(
                        yp[:, cc * 512:(cc + 1) * 512],
                        lhsT=ctxn[pr][:, ib * P:(ib + 1) * P],
                        rhs=wo_sb[:, pr, cc * 512:(cc + 1) * 512],
                        start=(pr == 0),
                        stop=(pr == NPAIR - 1),
                    )
            y_sb = work.tile([P, HID], bf16, name=f"y{ib}", tag="y", bufs=2)
            nc.scalar.activation(y_sb, yp, AF.Copy)
            nc.sync.dma_start(out=out[ib * P:(ib + 1) * P, :], in_=y_sb)

        normalize_mm(7, 0, cr_of[7][0])
        for ib in range(4):
            outproj(ib)
            if ib == 0:
                normalize_mm(7, 1, cr_of[7][1])
        for ib in range(4, NIB):
            outproj(ib)

    nc.compile()
    return nc


def _get_nc():
    global _CACHED_NC
    if _CACHED_NC is None:
        _CACHED_NC = _build_nc()
    return _CACHED_NC


def make_in_maps(q, k, v, attn_bias, Wq, Wk, Wv, Wo, bq, bk, bv, bo):
    in_maps = []
    for core in range(8):
        b, g = divmod(core, 2)
        gs = slice(g * GCOL, (g + 1) * GCOL)
        in_maps.append({
            "qT": np.ascontiguousarray(q[b].T).astype(BF16),
            "kT": np.ascontiguousarray(k[b].T).astype(BF16),
            "vT": np.ascontiguousarray(v[b].T).astype(BF16),
            "wq": np.ascontiguousarray(Wq[:, gs]).astype(BF16),
            "wk": np.ascontiguousarray(Wk[:, gs]).astype(BF16),
            "wv": np.ascontiguousarray(Wv[:, gs]).astype(BF16),
            "wo": np.ascontiguousarray(Wo[gs, :]).astype(BF16),
            "bq": np.ascontiguousarray(bq[gs]).astype(np.float32),
            "bk": np.ascontiguousarray(bk[gs]).astype(np.float32),
            "bv": np.ascontiguousarray(bv[gs]).astype(BF16),
            "expb": np.exp(
                attn_bias[b, g * 8:(g + 1) * 8].transpose(0, 2, 1)
            ).astype(BF16),
        })
    return in_maps


def kernel(q, k, v, attn_bias, Wq, Wk, Wv, Wo, bq, bk, bv, bo, _trace=False):
    from concourse.bass_utils import run_bass_kernel_spmd

    args = [np.asarray(x, dtype=np.float32) for x in
            (q, k, v, attn_bias, Wq, Wk, Wv, Wo, bq, bk, bv, bo)]
    q, k, v, attn_bias, Wq, Wk, Wv, Wo, bq, bk, bv, bo = args
    nc = _get_nc()
    in_maps = make_in_maps(q, k, v, attn_bias, Wq, Wk, Wv, Wo, bq, bk, bv, bo)
    res = run_bass_kernel_spmd(nc, in_maps, core_ids=list(range(8)), trace=_trace)
    y = np.zeros((4, S, HID), np.float32)
    for core in range(8):
        y[core // 2] += res.results[core]["out"].astype(np.float32)
    y += bo
    if _trace:
        kernel.last_results = res
    return y
